# revision 13
# baseline (speedup 1.0000x reference)
"""Raw-bass pipelined TT-linear kernel (v4).

Math: W (1024x1024) is a rank-20 TT product, so
  y = (x @ Hin) @ [Hout; bias] with Hin (1024,20), Hout (20,1024).
Data-parallel over batch: 8 cores x 2048 rows.

HW model measured from v1-v3 NTFF traces:
  - 16 shared SDMA engines; per-engine rate is packet-size bound
    (~8 KiB packets -> ~395 GB/s aggregate, 4 KiB -> ~310, 2 KiB ->
    ~220). HBM caps ~430 GB/s/core. So every bulk stream uses full
    16 KiB-per-partition channels (8 KiB packets).
  - Engines fair-share across active channels of all queues, so the
    output channels are issued on the two rings the inputs don't use.
  - The gpsimd SWDGE ring has ~4us first-transfer latency; a dummy
    64-byte DMA at the head warms it up.
  - Weights ride the scalar HWDGE ring concurrently with xt chunk 0 on
    the sync ring, so GEMM1(0) starts at ~11.5us.
  - Outputs go to DRAM in a [chunk, 128, 4096] layout (host
    de-transposes) so each out chunk is one 8 KiB-packet channel.
  - PSUM p1 is zeroed once at the head; GEMM1 uses start=True on each
    column group's first matmul (kc<4), so no per-chunk re-zeroing.
    Rows between the four 20-row group slices stay zero forever, so no
    NaN garbage reaches GEMM2 via t4 (houtb is zero there, but NaN*0
    would still poison it).  t4 rows 96-127 are memset to 1.0 once:
    row 116 is the bias/ones row, rows 96-115 are overwritten by every
    t4copy before GEMM2 reads them, rows 117-127 hit zero houtb rows.

Engine roles:
  sync   : the four xt chunk DMAs (nothing else competes with them)
  scalar : weights DMA, ACT warm-up, evac share, out DMAs c1, c3
  gpsimd : ring warm-up, t4 memsets, out DMAs c0, c2, final cleanup
  vector : p1 memsets, t4 group copies, evac share
  tensor : matmuls, software-pipelined G1(c+1) before G2(c)
"""

from contextlib import ExitStack

import numpy as np

import concourse.bass as bass
import concourse.mybir as mybir
from concourse.bass_utils import run_bass_kernel_spmd

N_CORES = 8
B_SHARD = 2048
D_IN = 1024
D_OUT = 1024
R = 20
KC = 8
CHUNK = 512
N_CHUNKS = B_SHARD // CHUNK
BT = CHUNK // 128
QPC = 2 * BT  # half-tiles per chunk
P2_BUFS = 6
BIAS_ROW = 116
HIN_COLS = KC * R  # 160
W_COLS = HIN_COLS + D_OUT  # hin ++ houtb

_DT = {"f32": mybir.dt.float32, "bf16": mybir.dt.bfloat16}

# evacuation engine per PAIR of half-tiles (pair = q//2): v=vector, s=scalar.
# Each evac op moves two adjacent PSUM banks (1024 f32 cols) in one
# instruction -- per-op overhead halves vs single-bank evacs.
_PEVAC = "vsvs"


def _eng(p):
    return _PEVAC[p % 4]


def _cnt(eng, p):
    """# of pairs with index <= p evacuated by `eng`."""
    return sum(1 for i in range(p + 1) if _eng(i) == eng)


def build_nc(compute="bf16", out_bf16=True):
    cdt = _DT[compute]
    odt = mybir.dt.bfloat16 if out_bf16 else mybir.dt.float32
    f32 = mybir.dt.float32

    nc = bass.Bass("TRN2", target_bir_lowering=False, debug=False)

    xt_d = nc.declare_dram_parameter(
        "xt", [N_CHUNKS, 128, KC * CHUNK], cdt, isOutput=False
    )
    wb_d = nc.declare_dram_parameter("wb", [128, W_COLS], cdt, isOutput=False)
    out_d = nc.declare_dram_parameter(
        "out", [N_CHUNKS, 128, BT * D_OUT], odt, isOutput=True
    )

    with ExitStack() as ctx:
        wb_sb = ctx.enter_context(nc.sbuf_tensor("wb_sb", [128, W_COLS], cdt))
        warm_sb = ctx.enter_context(nc.sbuf_tensor("warm_sb", [1, 64], cdt))
        xt_sb = [
            ctx.enter_context(nc.sbuf_tensor(f"xt{i}", [128, KC * CHUNK], cdt))
            for i in range(N_CHUNKS)
        ]
        t4_sb = [
            ctx.enter_context(nc.sbuf_tensor(f"t4{i}", [128, CHUNK], cdt))
            for i in range(2)
        ]
        y_sb = [
            ctx.enter_context(nc.sbuf_tensor(f"y{i}", [128, BT * D_OUT], odt))
            for i in range(N_CHUNKS)
        ]
        p1 = [
            ctx.enter_context(nc.psum_tensor(f"p1{i}", [128, 512], f32))
            for i in range(2)
        ]
        # p2: one 6-bank PSUM tensor; matmul q writes bank q%6, evacs move
        # bank pairs (0,1)/(2,3)/(4,5) -- (2p)%6 is always even, so pairs
        # never straddle a bank-pair boundary
        p2 = ctx.enter_context(
            nc.psum_tensor("p2", [128, P2_BUFS * 512], f32)
        )
        # DMA-completion semaphores: a dma_start's then_inc(sem, 16) is 16
        # independent +1s (one per SDMA engine), so only "all 16 done"
        # thresholds are race-free.
        sem_w = ctx.enter_context(nc.semaphore("sem_w"))
        sem_warm = ctx.enter_context(nc.semaphore("sem_warm"))
        sem_xtc = [
            ctx.enter_context(nc.semaphore(f"sem_xtc{i}"))
            for i in range(N_CHUNKS)
        ]
        sem_outc = [
            ctx.enter_context(nc.semaphore(f"sem_outc{i}"))
            for i in range(N_CHUNKS)
        ]
        (sem_mm1, sem_t4, sem_mm2, sem_yv, sem_ys, sem_p1z, sem_ones) = [
            ctx.enter_context(nc.semaphore(n))
            for n in (
                "sem_mm1", "sem_t4", "sem_mm2", "sem_yv", "sem_ys",
                "sem_p1z", "sem_ones",
            )
        ]
        sems = (
            [sem_w, sem_warm]
            + sem_xtc
            + sem_outc
            + [sem_mm1, sem_t4, sem_mm2, sem_yv, sem_ys, sem_p1z, sem_ones]
        )
        nums = sorted(s.num for s in sems)
        assert nums == list(range(nums[0], nums[0] + len(nums))), nums
        sem_range = range(nums[0], nums[-1] + 1)

        sem_of = {"v": sem_yv, "s": sem_ys}

        def evac_wait(engine, q):
            """Wait until the pair containing half-tile q is evacuated."""
            p = q // 2
            engine.wait_ge(sem_of[_eng(p)], _cnt(_eng(p), p))

        OC = 2 * D_OUT  # columns per output half-channel

        def out_dma(engine, c, h):
            p_last = 4 * c + 2 * h + 1
            for e in "vs":
                n = _cnt(e, p_last)
                if n:
                    engine.wait_ge(sem_of[e], n)
            engine.dma_start(
                out=out_d[c][:, h * OC : (h + 1) * OC],
                in_=y_sb[c][:, h * OC : (h + 1) * OC],
            ).then_inc(sem_outc[c], 16)

        with nc.Block() as block:

            @block.sync
            def _(sync):
                # weights lead on this ring: on the gpsimd ring their
                # small (2.4 KiB row) packets crawl and dilute the input
                # stream (v6: hout took 9->17.3us there); here they cost
                # ~1.4us of ring head and land by ~10.3us (v5 measured)
                sync.dma_start(out=wb_sb[:], in_=wb_d[:]).then_inc(sem_w, 16)
                for c in range(N_CHUNKS):
                    sync.dma_start(out=xt_sb[c][:], in_=xt_d[c]).then_inc(
                        sem_xtc[c], 16
                    )
                for c in range(N_CHUNKS):
                    out_dma(sync, c, 0)

            @block.tensor
            def _(tensor):
                def g1(c):
                    # four column groups concurrent (tile_position=(0,32j));
                    # kc<4 overwrites (start=True), kc>=4 accumulates
                    for kc in range(KC):
                        j = kc % 4
                        if kc == 0:
                            if c == 0:
                                tensor.wait_ge(sem_w, 16)
                            tensor.wait_ge(sem_xtc[c], 16)
                            if c < 2:
                                tensor.wait_ge(sem_p1z, c + 1)
                            else:
                                # start=True overwrite must not race
                                # t4copy(c-2)'s read of this bank
                                tensor.wait_ge(sem_t4, c - 1)
                        mm = tensor.matmul(
                            p1[c % 2][32 * j : 32 * j + R, 0:CHUNK],
                            wb_sb[:, kc * R : (kc + 1) * R],
                            xt_sb[c][:, kc * CHUNK : (kc + 1) * CHUNK],
                            start=(kc < 4),
                            stop=(kc == KC - 1),
                            tile_position=(0, 32 * j),
                            skip_group_check=True,
                        )
                        if kc == KC - 1:
                            mm.then_inc(sem_mm1)

                def g2(c):
                    for bt in range(BT):
                        for nh in range(2):
                            q = QPC * c + 2 * bt + nh
                            if q == QPC * c:
                                tensor.wait_ge(sem_t4, c + 1)
                                if c < 2:
                                    tensor.wait_ge(sem_ones, c + 1)
                            if q >= P2_BUFS:
                                evac_wait(tensor, q - P2_BUFS)
                            tensor.matmul(
                                p2[:, (q % P2_BUFS) * 512 : (q % P2_BUFS) * 512 + 512],
                                t4_sb[c % 2][:, bt * 128 : (bt + 1) * 128],
                                wb_sb[
                                    :,
                                    HIN_COLS + nh * 512 : HIN_COLS + (nh + 1) * 512,
                                ],
                                start=True,
                                stop=True,
                            ).then_inc(sem_mm2)

                # software pipeline: g1(c+1) issues before g2(c), so the
                # t4copy(c) latency hides under g1(c+1) instead of gating
                # the tensor queue (v5 lost ~1.2us/chunk to this gap)
                g1(0)
                for c in range(N_CHUNKS):
                    if c + 1 < N_CHUNKS:
                        g1(c + 1)
                    g2(c)

            @block.vector
            def _(vector):
                # one-time p1 zeroing: group-gap rows must stay exactly 0
                # (PSUM garbage could be NaN; NaN*0 poisons GEMM2)
                vector.memset(p1[0][:], 0.0).then_inc(sem_p1z)
                vector.memset(p1[1][:], 0.0).then_inc(sem_p1z)

                def t4copy(c):
                    vector.wait_ge(sem_mm1, c + 1)
                    if c < 2:
                        vector.wait_ge(sem_ones, c + 1)  # head memsets done
                    else:
                        # t4 buffer reuse: all GEMM2 of chunk c-2 done
                        vector.wait_ge(sem_mm2, QPC * (c - 2) + QPC)
                    vector.tensor_copy(
                        t4_sb[c % 2][0:BIAS_ROW, :],
                        p1[c % 2][0:BIAS_ROW, 0:CHUNK],
                    ).then_inc(sem_t4)

                def evacs(c):
                    for bt in range(BT):
                        p = 4 * c + bt
                        if _eng(p) != "v":
                            continue
                        vector.wait_ge(sem_mm2, 2 * p + 2)
                        b0 = ((2 * p) % P2_BUFS) * 512
                        vector.tensor_copy(
                            y_sb[c][:, bt * D_OUT : (bt + 1) * D_OUT],
                            p2[:, b0 : b0 + 1024],
                        ).then_inc(sem_yv)

                t4copy(0)
                for c in range(N_CHUNKS):
                    if c + 1 < N_CHUNKS:
                        t4copy(c + 1)
                    evacs(c)

            @block.scalar
            def _(scalar):
                # dummy copy: pull the one-time ACT_TABLE_LOAD (~1.3us) into
                # the head instead of the first real evacuation
                scalar.wait_ge(sem_ones, 1)
                scalar.copy(y_sb[0][0:1, 0:32], t4_sb[0][0:1, 0:32])
                for c in range(N_CHUNKS):
                    for bt in range(BT):
                        p = 4 * c + bt
                        if _eng(p) != "s":
                            continue
                        scalar.wait_ge(sem_mm2, 2 * p + 2)
                        b0 = ((2 * p) % P2_BUFS) * 512
                        scalar.copy(
                            y_sb[c][:, bt * D_OUT : (bt + 1) * D_OUT],
                            p2[:, b0 : b0 + 1024],
                        ).then_inc(sem_ys)

            @block.gpsimd
            def _(gpsimd):
                # warm up the SWDGE ring now so out c3 (~27us) doesn't pay
                # the ~4us first-transfer latency
                gpsimd.dma_start(out=warm_sb[:], in_=wb_d[0:1, 0:64]).then_inc(
                    sem_warm, 16
                )
                # t4 rows 96-127 <- 1.0 once (partition base must be 32-
                # aligned): row 116 is the bias/ones row; rows 96-115 are
                # re-written by every t4copy before GEMM2 reads them; rows
                # 117-127 hit zero houtb rows (1.0, not garbage, so no NaN).
                for i in range(2):
                    gpsimd.memset(t4_sb[i][96:128, :], 1.0).then_inc(sem_ones)
                for c in range(N_CHUNKS):
                    out_dma(gpsimd, c, 1)
                for c in range(N_CHUNKS):
                    gpsimd.wait_ge(sem_outc[c], 32)
                # leave semaphores clean for any re-execution
                gpsimd.dma_reset(sem_range)
                gpsimd.sem_clear(sem_range)

    return nc


def host_prep(x, cores, bias, np_dt):
    A = cores[0][0].astype(np.float64)
    for G in cores[1:4]:
        G = G.astype(np.float64)
        A = np.einsum("ir,rjs->ijs", A, G).reshape(-1, G.shape[2])
    H = cores[4].astype(np.float64)
    for G in cores[5:]:
        G = G.astype(np.float64)
        H = np.einsum("pNq,qnr->pNnr", H, G).reshape(H.shape[0], -1, G.shape[2])
    H = H.reshape(H.shape[0], -1)  # (20, 1024)

    hin = np.ascontiguousarray(
        A.reshape(KC, 128, R).transpose(1, 0, 2).reshape(128, KC * R)
    )
    # Hout replicated into the four 32-row column groups + bias in row 116;
    # rows outside the rank blocks stay exactly 0 (t4 garbage protection)
    houtb = np.zeros((128, D_OUT), dtype=np.float64)
    for j in range(4):
        houtb[32 * j : 32 * j + R, :] = H
    houtb[BIAS_ROW, :] = bias.astype(np.float64)
    wb = np.concatenate([hin, houtb], axis=1).astype(np_dt)  # [128, 1184]
    xt = np.ascontiguousarray(
        x.reshape(N_CORES, N_CHUNKS, CHUNK, KC, 128).transpose(0, 1, 4, 3, 2)
    ).astype(np_dt).reshape(N_CORES, N_CHUNKS, 128, KC * CHUNK)
    return xt, wb


def unshard_out(raw):
    """[N_CHUNKS, 128, BT*D_OUT] -> [B_SHARD, D_OUT]"""
    return (
        raw.reshape(N_CHUNKS, 128, BT, D_OUT)
        .transpose(0, 2, 1, 3)
        .reshape(B_SHARD, D_OUT)
    )


_NC_CACHE = {}


def run(x, cores, bias, compute="bf16", out_bf16=True, trace=False):
    np_dt = np.dtype(mybir.dt.np(_DT[compute]))
    xt, wb = host_prep(x, cores, bias, np_dt)
    key = (compute, out_bf16)
    if key not in _NC_CACHE:
        _NC_CACHE[key] = build_nc(compute, out_bf16)
    nc = _NC_CACHE[key]
    in_maps = [{"xt": xt[i], "wb": wb} for i in range(N_CORES)]
    res = run_bass_kernel_spmd(nc, in_maps, list(range(N_CORES)), trace=trace)
    out = np.concatenate(
        [unshard_out(res.results[i]["out"]) for i in range(N_CORES)], axis=0
    )
    return out.astype(np.float32), res


def kernel(x, core0, core1, core2, core3, core4, core5, core6, core7, bias):
    cores = (core0, core1, core2, core3, core4, core5, core6, core7)
    out, _ = run(
        np.asarray(x, dtype=np.float32),
        [np.asarray(c, dtype=np.float32) for c in cores],
        np.asarray(bias, dtype=np.float32),
    )
    return out


# revision 15
# speedup vs baseline: 1.0126x; 1.0126x over previous
"""Raw-bass pipelined TT-linear kernel (v9).

Math: W (1024x1024) is a rank-20 TT product, so
  y = (x @ Hin) @ [Hout; bias] with Hin (1024,20), Hout (20,1024).
Data-parallel over batch: 8 cores x 2048 rows.

HW model measured from the v1-v8 NTFF traces:
  - 16 shared SDMA engines, packet-size-bound (~8 KiB packets ->
    ~400 GB/s aggregate, 4 KiB -> ~310); read-only streams reach
    ~430-446 GB/s but WRITE-ONLY streams only ~250-300 GB/s, while
    mixed read+write totals ~440. So the schedule front-loads small
    chunks to get outputs flowing ~13us (overlapping the read stream
    on the otherwise-idle gpsimd ring) instead of paying a 14us
    write-only tail.
  - Engines fair-share across active channels of all queues; weights
    lead on the sync ring (tiny 2.4 KiB-row packets crawl anywhere
    else) and outputs never queue ahead of inputs on the sync ring.
  - The gpsimd SWDGE ring has ~4us first-transfer latency; its first
    (warm-up) DMA hides it.
  - Chunks: [256, 256, 512, 512, 512] rows. Small leading chunks start
    compute ~12us and the first write ~13.5us; 512-row steady chunks
    keep 8 KiB input packets.
  - PSUM p1 is zeroed once at the head; GEMM1 uses start=True on each
    column group's first matmul (kc<4). Rows between the four 20-row
    group slices stay zero forever, so no NaN garbage reaches GEMM2
    via t4 (houtb is zero there, but NaN*0 would still poison it).
    t4 rows 96-127 are memset to 1.0 once: row 116 is the bias/ones
    row, rows 96-115 are overwritten by every t4copy before GEMM2
    reads them, rows 117-127 hit zero houtb rows.
  - PSUM p2 is one 6-bank tensor; evacuation moves bank PAIRS
    (1024 f32 cols) per op, alternating DVE/ACT globally.

Engine roles:
  sync   : weights, the five xt chunk DMAs, late output channels
  gpsimd : ring warm-up, t4 memsets, early output channels, cleanup
  scalar : ACT warm-up + evac share
  vector : p1 memsets, t4 group copies, evac share
  tensor : matmuls, software-pipelined G1(c+1) before G2(c)
"""

from contextlib import ExitStack

import numpy as np

import concourse.bass as bass
import concourse.mybir as mybir
from concourse.bass_utils import run_bass_kernel_spmd

N_CORES = 8
B_SHARD = 2048
D_IN = 1024
D_OUT = 1024
R = 20
KC = 8
BIAS_ROW = 116
HIN_COLS = KC * R  # 160
W_COLS = HIN_COLS + D_OUT  # hin ++ houtb
P2_BANKS = 6

CHUNKS = [256, 256, 512, 512, 512]  # rows per chunk (sum = B_SHARD)
N_CHUNKS = len(CHUNKS)
OFFS = [sum(CHUNKS[:i]) for i in range(N_CHUNKS)]  # row offsets
BTS = [r // 128 for r in CHUNKS]  # 128-row tiles per chunk
# cumulative GEMM2 matmul counts / evac-pair counts at chunk end
MM2C = [sum(2 * b for b in BTS[: i + 1]) for i in range(N_CHUNKS)]
PRC = [sum(BTS[: i + 1]) for i in range(N_CHUNKS)]

assert sum(CHUNKS) == B_SHARD

_DT = {"f32": mybir.dt.float32, "bf16": mybir.dt.bfloat16}

# evacuation engine per global pair (pair = one 128-row bt tile, two
# GEMM2 matmuls / two adjacent PSUM banks): v=vector, s=scalar
_PEVAC = "vs"


def _eng(p):
    return _PEVAC[p % 2]


def _cnt(eng, p):
    """# of pairs with index <= p evacuated by `eng`."""
    return sum(1 for i in range(p + 1) if _eng(i) == eng)


# output channels: (chunk, bt_start, n_bt); <=2 tiles (512 KiB) each
OUT_CHANNELS = []
for _c in range(N_CHUNKS):
    for _b in range(0, BTS[_c], 2):
        OUT_CHANNELS.append((_c, _b, min(2, BTS[_c] - _b)))
# the earliest-ready channels ride the gpsimd ring so they overlap the
# read stream (writes during the read phase are nearly free bandwidth);
# later ones alternate so both rings carry the post-input write drain
_GPS_IDX = {0, 1, 3, 5, 7}
GPS_CHANNELS = [ch for i, ch in enumerate(OUT_CHANNELS) if i in _GPS_IDX]
SYNC_CHANNELS = [ch for i, ch in enumerate(OUT_CHANNELS) if i not in _GPS_IDX]
# per-chunk expected outc increments (16 per channel)
OUTC_TOTAL = [16 * sum(1 for ch in OUT_CHANNELS if ch[0] == c)
              for c in range(N_CHUNKS)]


def build_nc(compute="bf16", out_bf16=True):
    cdt = _DT[compute]
    odt = mybir.dt.bfloat16 if out_bf16 else mybir.dt.float32
    f32 = mybir.dt.float32

    nc = bass.Bass("TRN2", target_bir_lowering=False, debug=False)

    xt_d = nc.declare_dram_parameter(
        "xt", [128, KC * B_SHARD], cdt, isOutput=False
    )
    wb_d = nc.declare_dram_parameter("wb", [128, W_COLS], cdt, isOutput=False)
    out_d = nc.declare_dram_parameter(
        "out", [128, B_SHARD // 128 * D_OUT], odt, isOutput=True
    )

    def xt_dcols(c):
        return slice(KC * OFFS[c], KC * (OFFS[c] + CHUNKS[c]))

    def out_dcols(c, b, nb):
        o0 = (OFFS[c] // 128 + b) * D_OUT
        return slice(o0, o0 + nb * D_OUT)

    with ExitStack() as ctx:
        wb_sb = ctx.enter_context(nc.sbuf_tensor("wb_sb", [128, W_COLS], cdt))
        warm_sb = ctx.enter_context(nc.sbuf_tensor("warm_sb", [1, 64], cdt))
        xt_sb = [
            ctx.enter_context(
                nc.sbuf_tensor(f"xt{i}", [128, KC * CHUNKS[i]], cdt)
            )
            for i in range(N_CHUNKS)
        ]
        t4_sb = [
            ctx.enter_context(nc.sbuf_tensor(f"t4{i}", [128, 512], cdt))
            for i in range(2)
        ]
        y_sb = [
            ctx.enter_context(
                nc.sbuf_tensor(f"y{i}", [128, BTS[i] * D_OUT], odt)
            )
            for i in range(N_CHUNKS)
        ]
        p1 = [
            ctx.enter_context(nc.psum_tensor(f"p1{i}", [128, 512], f32))
            for i in range(2)
        ]
        # one 6-bank PSUM tensor; matmul q writes bank q%6, evacs move
        # bank pairs (0,1)/(2,3)/(4,5) -- (2p)%6 is always even
        p2 = ctx.enter_context(
            nc.psum_tensor("p2", [128, P2_BANKS * 512], f32)
        )
        sem_w = ctx.enter_context(nc.semaphore("sem_w"))
        sem_warm = ctx.enter_context(nc.semaphore("sem_warm"))
        sem_xtc = [
            ctx.enter_context(nc.semaphore(f"sem_xtc{i}"))
            for i in range(N_CHUNKS)
        ]
        sem_outc = [
            ctx.enter_context(nc.semaphore(f"sem_outc{i}"))
            for i in range(N_CHUNKS)
        ]
        (sem_mm1, sem_t4, sem_mm2, sem_yv, sem_ys, sem_p1z, sem_ones) = [
            ctx.enter_context(nc.semaphore(n))
            for n in (
                "sem_mm1", "sem_t4", "sem_mm2", "sem_yv", "sem_ys",
                "sem_p1z", "sem_ones",
            )
        ]
        sems = (
            [sem_w, sem_warm]
            + sem_xtc
            + sem_outc
            + [sem_mm1, sem_t4, sem_mm2, sem_yv, sem_ys, sem_p1z, sem_ones]
        )
        nums = sorted(s.num for s in sems)
        assert nums == list(range(nums[0], nums[0] + len(nums))), nums
        sem_range = range(nums[0], nums[-1] + 1)

        sem_of = {"v": sem_yv, "s": sem_ys}

        def evac_wait(engine, q):
            """Wait until the pair containing GEMM2 matmul q is evacuated."""
            p = q // 2
            engine.wait_ge(sem_of[_eng(p)], _cnt(_eng(p), p))

        def out_dma(engine, ch):
            c, b, nb = ch
            p_last = (PRC[c - 1] if c else 0) + b + nb - 1
            for e in "vs":
                n = _cnt(e, p_last)
                if n:
                    engine.wait_ge(sem_of[e], n)
            engine.dma_start(
                out=out_d[:, out_dcols(c, b, nb)],
                in_=y_sb[c][:, b * D_OUT : (b + nb) * D_OUT],
            ).then_inc(sem_outc[c], 16)

        with nc.Block() as block:

            @block.sync
            def _(sync):
                # weights lead: their small (2.4 KiB-row) packets crawl on
                # any ring that competes with bulk 8 KiB streams
                sync.dma_start(out=wb_sb[:], in_=wb_d[:]).then_inc(sem_w, 16)
                for c in range(N_CHUNKS):
                    sync.dma_start(
                        out=xt_sb[c][:], in_=xt_d[:, xt_dcols(c)]
                    ).then_inc(sem_xtc[c], 16)
                for ch in SYNC_CHANNELS:
                    out_dma(sync, ch)

            @block.tensor
            def _(tensor):
                def g1(c):
                    rows = CHUNKS[c]
                    for kc in range(KC):
                        j = kc % 4
                        if kc == 0:
                            if c == 0:
                                tensor.wait_ge(sem_w, 16)
                            tensor.wait_ge(sem_xtc[c], 16)
                            if c < 2:
                                tensor.wait_ge(sem_p1z, c + 1)
                            else:
                                # start=True overwrite must not race
                                # t4copy(c-2)'s read of this bank
                                tensor.wait_ge(sem_t4, c - 1)
                        mm = tensor.matmul(
                            p1[c % 2][32 * j : 32 * j + R, 0:rows],
                            wb_sb[:, kc * R : (kc + 1) * R],
                            xt_sb[c][:, kc * rows : (kc + 1) * rows],
                            start=(kc < 4),
                            stop=(kc == KC - 1),
                            tile_position=(0, 32 * j),
                            skip_group_check=True,
                        )
                        if kc == KC - 1:
                            mm.then_inc(sem_mm1)

                def g2(c):
                    q0 = MM2C[c - 1] if c else 0
                    for bt in range(BTS[c]):
                        for nh in range(2):
                            q = q0 + 2 * bt + nh
                            if q == q0:
                                tensor.wait_ge(sem_t4, c + 1)
                                if c < 2:
                                    tensor.wait_ge(sem_ones, c + 1)
                            if q >= P2_BANKS:
                                evac_wait(tensor, q - P2_BANKS)
                            b0 = (q % P2_BANKS) * 512
                            tensor.matmul(
                                p2[:, b0 : b0 + 512],
                                t4_sb[c % 2][:, bt * 128 : (bt + 1) * 128],
                                wb_sb[
                                    :,
                                    HIN_COLS + nh * 512 : HIN_COLS + (nh + 1) * 512,
                                ],
                                start=True,
                                stop=True,
                            ).then_inc(sem_mm2)

                # software pipeline: g1(c+1) issues before g2(c), so the
                # t4copy(c) latency hides under g1(c+1)
                g1(0)
                for c in range(N_CHUNKS):
                    if c + 1 < N_CHUNKS:
                        g1(c + 1)
                    g2(c)

            @block.vector
            def _(vector):
                # one-time p1 zeroing: group-gap rows must stay exactly 0
                # (PSUM garbage could be NaN; NaN*0 poisons GEMM2)
                vector.memset(p1[0][:], 0.0).then_inc(sem_p1z)
                vector.memset(p1[1][:], 0.0).then_inc(sem_p1z)

                def t4copy(c):
                    vector.wait_ge(sem_mm1, c + 1)
                    if c < 2:
                        vector.wait_ge(sem_ones, c + 1)  # head memsets done
                    else:
                        # t4 buffer reuse: all GEMM2 of chunk c-2 done
                        vector.wait_ge(sem_mm2, MM2C[c - 2])
                    vector.tensor_copy(
                        t4_sb[c % 2][0:BIAS_ROW, 0 : CHUNKS[c]],
                        p1[c % 2][0:BIAS_ROW, 0 : CHUNKS[c]],
                    ).then_inc(sem_t4)

                def evacs(c):
                    for bt in range(BTS[c]):
                        p = (PRC[c - 1] if c else 0) + bt
                        if _eng(p) != "v":
                            continue
                        vector.wait_ge(sem_mm2, 2 * p + 2)
                        b0 = ((2 * p) % P2_BANKS) * 512
                        vector.tensor_copy(
                            y_sb[c][:, bt * D_OUT : (bt + 1) * D_OUT],
                            p2[:, b0 : b0 + 1024],
                        ).then_inc(sem_yv)

                t4copy(0)
                for c in range(N_CHUNKS):
                    if c + 1 < N_CHUNKS:
                        t4copy(c + 1)
                    evacs(c)

            @block.scalar
            def _(scalar):
                # dummy copy: pull the one-time ACT_TABLE_LOAD (~1.3us) into
                # the head instead of the first real evacuation
                scalar.wait_ge(sem_ones, 1)
                scalar.copy(y_sb[0][0:1, 0:32], t4_sb[0][0:1, 0:32])
                for c in range(N_CHUNKS):
                    for bt in range(BTS[c]):
                        p = (PRC[c - 1] if c else 0) + bt
                        if _eng(p) != "s":
                            continue
                        scalar.wait_ge(sem_mm2, 2 * p + 2)
                        b0 = ((2 * p) % P2_BANKS) * 512
                        scalar.copy(
                            y_sb[c][:, bt * D_OUT : (bt + 1) * D_OUT],
                            p2[:, b0 : b0 + 1024],
                        ).then_inc(sem_ys)

            @block.gpsimd
            def _(gpsimd):
                # warm up the SWDGE ring now (first transfer pays ~4us)
                gpsimd.dma_start(out=warm_sb[:], in_=wb_d[0:1, 0:64]).then_inc(
                    sem_warm, 16
                )
                # t4 rows 96-127 <- 1.0 once (partition base must be 32-
                # aligned): row 116 is the bias/ones row; rows 96-115 are
                # re-written by every t4copy before GEMM2 reads them; rows
                # 117-127 hit zero houtb rows (1.0, not garbage, so no NaN).
                for i in range(2):
                    gpsimd.memset(t4_sb[i][96:128, :], 1.0).then_inc(sem_ones)
                for ch in GPS_CHANNELS:
                    out_dma(gpsimd, ch)
                for c in range(N_CHUNKS):
                    gpsimd.wait_ge(sem_outc[c], OUTC_TOTAL[c])
                # leave semaphores clean for any re-execution
                gpsimd.dma_reset(sem_range)
                gpsimd.sem_clear(sem_range)

    return nc


def host_prep(x, cores, bias, np_dt):
    A = cores[0][0].astype(np.float64)
    for G in cores[1:4]:
        G = G.astype(np.float64)
        A = np.einsum("ir,rjs->ijs", A, G).reshape(-1, G.shape[2])
    H = cores[4].astype(np.float64)
    for G in cores[5:]:
        G = G.astype(np.float64)
        H = np.einsum("pNq,qnr->pNnr", H, G).reshape(H.shape[0], -1, G.shape[2])
    H = H.reshape(H.shape[0], -1)  # (20, 1024)

    hin = np.ascontiguousarray(
        A.reshape(KC, 128, R).transpose(1, 0, 2).reshape(128, KC * R)
    )
    # Hout replicated into the four 32-row column groups + bias in row 116;
    # rows outside the rank blocks stay exactly 0 (t4 garbage protection)
    houtb = np.zeros((128, D_OUT), dtype=np.float64)
    for j in range(4):
        houtb[32 * j : 32 * j + R, :] = H
    houtb[BIAS_ROW, :] = bias.astype(np.float64)
    wb = np.concatenate([hin, houtb], axis=1).astype(np_dt)  # [128, 1184]

    xs = x.reshape(N_CORES, B_SHARD, D_IN)
    blocks = []
    for c in range(N_CHUNKS):
        blk = xs[:, OFFS[c] : OFFS[c] + CHUNKS[c], :]
        blk = blk.reshape(N_CORES, CHUNKS[c], KC, 128).transpose(0, 3, 2, 1)
        blocks.append(blk.reshape(N_CORES, 128, KC * CHUNKS[c]))
    xt = np.ascontiguousarray(np.concatenate(blocks, axis=2)).astype(np_dt)
    return xt, wb


def unshard_out(raw):
    """[128, (B_SHARD/128)*D_OUT] -> [B_SHARD, D_OUT]"""
    nb = B_SHARD // 128
    return (
        raw.reshape(128, nb, D_OUT).transpose(1, 0, 2).reshape(B_SHARD, D_OUT)
    )


_NC_CACHE = {}


def run(x, cores, bias, compute="bf16", out_bf16=True, trace=False):
    np_dt = np.dtype(mybir.dt.np(_DT[compute]))
    xt, wb = host_prep(x, cores, bias, np_dt)
    key = (compute, out_bf16)
    if key not in _NC_CACHE:
        _NC_CACHE[key] = build_nc(compute, out_bf16)
    nc = _NC_CACHE[key]
    in_maps = [{"xt": xt[i], "wb": wb} for i in range(N_CORES)]
    res = run_bass_kernel_spmd(nc, in_maps, list(range(N_CORES)), trace=trace)
    out = np.concatenate(
        [unshard_out(res.results[i]["out"]) for i in range(N_CORES)], axis=0
    )
    return out.astype(np.float32), res


def kernel(x, core0, core1, core2, core3, core4, core5, core6, core7, bias):
    cores = (core0, core1, core2, core3, core4, core5, core6, core7)
    out, _ = run(
        np.asarray(x, dtype=np.float32),
        [np.asarray(c, dtype=np.float32) for c in cores],
        np.asarray(bias, dtype=np.float32),
    )
    return out
